# revision 23
# baseline (speedup 1.0000x reference)
"""Trainium2 Bass kernel for nn_DecoderRNN (teacher-forced GRU decoder).

Strategy (8 NeuronCores):
  - Vocab-tensor-parallel output projection: out_W/out_b sharded 4000 rows/core;
    each core computes logits[:, c*4000:(c+1)*4000] for all B*T rows plus a
    local sum(exp(logit)) per row; AllReduce(add) combines the log_softmax
    normalizer (split in two stages so the f32 writeout overlaps the second
    half's matmuls); each core writes its f32 log_prob slice.
  - GRU recurrence + attention are replicated on every core (the recurrent
    matmul is weight-load bound, so batch sharding would not speed it up, and
    every core needs all B*T hidden states for its vocab slice anyway).
  - All big matmuls run in bf16 (measured end-to-end rel-err ~3e-3), with f32
    hidden state carried between steps and f32 attention scores/softmax.

Device layout: everything "transposed dense" — feature dims on the 128 SBUF
partitions, (batch*time) along the free axis, so the per-step GRU gate math
runs full-width DVE/ACT ops of shape [128, 64]. All inputs are pre-arranged
on the host into partition-major [128, ...] blocks so every input DMA is a
single fully-contiguous per-partition read.
"""

import sys
import numpy as np
import ml_dtypes

sys.path.insert(0, "/opt/trn_rl_repo")

import concourse.bass as bass
import concourse.bacc as bacc
import concourse.mybir as mybir
import concourse.tile as tile
from concourse import bass_utils
from concourse.bass_utils import run_bass_kernel_spmd
from concourse.masks import make_identity

FP32 = mybir.dt.float32
FP16 = mybir.dt.float16
AF = mybir.ActivationFunctionType
ALU = mybir.AluOpType
AX = mybir.AxisListType

B, T, S, H, V = 16, 64, 128, 512, 32000
NCORES = 8
VS = V // NCORES           # 4000 vocab rows per core
BT = B * T                 # 1024
NK = H // 128              # 4   k-tiles of the hidden dim
NM = 3 * H // 128          # 12  m-tiles of the gate dim
RT = BT // 128             # 8   row-tiles of B*T
NVC = 8                    # vocab chunks per core
VC = VS // NVC             # 500 columns per matmul chunk

f16 = np.float16

_ENABLE_LDW_OPT = False


def _patch_walrus_flags():
    """Flip --enable-ldw-opt so LDWEIGHTS runs with fast-weight-load."""
    orig = bass_utils.run_command
    if getattr(orig, "_ldw_patched", False):
        return

    def patched(cmd, *a, **kw):
        if _ENABLE_LDW_OPT and isinstance(cmd, list):
            cmd = ["--enable-ldw-opt=true" if c == "--enable-ldw-opt=false"
                   else c for c in cmd]
        return orig(cmd, *a, **kw)

    patched._ldw_patched = True
    bass_utils.run_command = patched


def _build_program():
    nc = bacc.Bacc("TRN2", target_bir_lowering=False, debug=False,
                   num_devices=NCORES)

    d_xT = nc.dram_tensor("xT", [128, NK, BT], FP16, kind="ExternalInput")
    d_wih = nc.dram_tensor("wih", [128, NK, 3 * H], FP16, kind="ExternalInput")
    d_whh = nc.dram_tensor("whh", [128, NK, 3 * H], FP16, kind="ExternalInput")
    # bias cols: 0:8 = (b_ih+b_hh)[:1024] tiles, 8:12 = b_ih[1024:], 12:16 = attn_b
    d_bias = nc.dram_tensor("bias", [128, 16], FP32, kind="ExternalInput")
    d_bnw = nc.dram_tensor("bnw", [128, NK, B], FP32, kind="ExternalInput")
    d_h0T = nc.dram_tensor("h0T", [128, NK, B], FP32, kind="ExternalInput")
    d_encT = nc.dram_tensor("encT", [128, B, NK, S], FP16, kind="ExternalInput")
    d_enc = nc.dram_tensor("enc", [128, B, H], FP16, kind="ExternalInput")
    d_awT = nc.dram_tensor("awT", [128, 2 * NK, H], FP16, kind="ExternalInput")
    d_owT = nc.dram_tensor("owT", [128, NK, VS], FP16, kind="ExternalInput")
    d_ob = nc.dram_tensor("ob", [128, VS], FP16, kind="ExternalInput")

    d_lp = nc.dram_tensor("lp", [BT, VS], FP32, kind="ExternalOutput")
    d_attn = nc.dram_tensor("attn", [T, B, S], FP32, kind="ExternalOutput")
    d_hl = nc.dram_tensor("hl", [128, NK, B], FP32, kind="ExternalOutput")

    with tile.TileContext(nc) as tc:
        _body(tc, nc, d_xT, d_wih, d_whh, d_bias, d_bnw, d_h0T, d_encT,
              d_enc, d_awT, d_owT, d_ob, d_lp, d_attn, d_hl)

    nc.compile()
    return nc


def _body(tc, nc, d_xT, d_wih, d_whh, d_bias, d_bnw, d_h0T, d_encT, d_enc,
          d_awT, d_owT, d_ob, d_lp, d_attn, d_hl):
    # Two SBUF allocation stacks: left holds the GRU-phase tensors (freed in
    # LIFO order as phases retire), right holds the late-phase weights and
    # the tensors that survive into the logits phase.
    with tc.tile_pool(name="persist", bufs=1) as per, \
         tc.tile_pool(name="dram", bufs=1, space="DRAM") as dramp, \
         tc.tile_pool(name="bw1", bufs=1, side="right") as bw1, \
         tc.tile_pool(name="bw2", bufs=1, side="right") as bw2:

        biasT = per.tile([128, 16], FP32)
        nc.sync.dma_start(biasT, d_bias.ap())
        bnw = per.tile([128, NK, B], FP32)
        nc.sync.dma_start(bnw, d_bnw.ap())
        h0f = per.tile([128, NK, B], FP32)
        nc.sync.dma_start(h0f, d_h0T.ap())
        ident = per.tile([128, 128], FP32)
        make_identity(nc, ident)

        # tiny warmup collective: pays the cold ncfw/CC-path cost during the
        # GRU so the real normalizer AllReduces run at the warm floor
        wu_in = dramp.tile([128, 1], FP32)
        wu_out = dramp.tile([128, 1], FP32)
        wu_s = per.tile([128, 1], FP32)
        nc.any.memset(wu_s, 0.0)
        nc.sync.dma_start(wu_in, wu_s)
        nc.gpsimd.collective_compute(
            "AllReduce", ALU.add,
            replica_groups=[list(range(NCORES))],
            ins=[wu_in.opt()], outs=[wu_out.opt()])

        awT = bw1.tile([128, 2 * NK, H], FP16)
        nc.sync.dma_start(awT, d_awT.ap())
        ob = bw1.tile([128, VS], FP16)
        nc.sync.dma_start(ob, d_ob.ap())

        with tc.tile_pool(name="encp", bufs=1) as ep:
            encT = ep.tile([128, B, NK, S], FP16)
            nc.scalar.dma_start(encT, d_encT.ap())

            with tc.tile_pool(name="outs", bufs=1) as ot:
                # hidden states for all steps, transposed dense: [p, k, b, t]
                outsT = ot.tile([128, NK, B, T], FP32)

                with tc.tile_pool(name="p12", bufs=1) as p12:
                    xgT = p12.tile([128, NM, B, T], FP32)
                    whh = p12.tile([128, NK, 3 * H], FP16)

                    # ------------ P1: xgT = W_ih @ x.T (+ biases) ---------
                    with tc.tile_pool(name="gw1", bufs=1) as gw1, \
                         tc.tile_pool(name="ps1", bufs=4, space="PSUM") as ps1:
                        xT = gw1.tile([128, NK, BT], FP16)
                        wih = gw1.tile([128, NK, 3 * H], FP16)
                        for k in range(NK):
                            nc.sync.dma_start(wih[:, k, :], d_wih.ap()[:, k, :])
                            nc.sync.dma_start(xT[:, k, :], d_xT.ap()[:, k, :])
                        nc.sync.dma_start(whh, d_whh.ap())

                        for m in range(NM):
                            bcol = m if m < 8 else 8 + (m - 8)
                            pshalf = [ps1.tile([128, 512], FP32, tag="xg",
                                               name=f"xg{m}_{h_}")
                                      for h_ in range(2)]
                            for k in range(NK):
                                for half in range(2):
                                    nc.tensor.matmul(
                                        pshalf[half],
                                        lhsT=wih[:, k, bass.ts(m, 128)],
                                        rhs=xT[:, k, bass.ts(half, 512)],
                                        start=(k == 0), stop=(k == NK - 1))
                            for half in range(2):
                                dst = xgT[:, m, 8 * half:8 * (half + 1), :]
                                nc.scalar.activation(
                                    dst,
                                    pshalf[half].rearrange("p (b t) -> p b t",
                                                           t=T),
                                    AF.Identity,
                                    bias=biasT[:, bcol:bcol + 1])

                    # late-phase weights: load during the GRU
                    owT = bw2.tile([128, NK, VS], FP16)
                    nc.sync.dma_start(owT, d_owT.ap())
                    outs16 = bw2.tile([128, NK, B, T], FP16)
                    mixT = bw2.tile([128, NK, B, T], FP16)
                    out2T = bw2.tile([128, NK, B, T], FP16)

                    # ------------ P2: GRU recurrence ----------------------
                    # emission order r(0:4), n(8:12), z(4:8); h' = n + z*(h-n)
                    m_order = [0, 1, 2, 3, 8, 9, 10, 11, 4, 5, 6, 7]
                    with tc.tile_pool(name="gru", bufs=3) as gp, \
                         tc.tile_pool(name="hb", bufs=2) as hp, \
                         tc.tile_pool(name="psg", bufs=2, space="PSUM") as psg:
                        hbf = hp.tile([128, NK, B], FP16, tag="hbf")
                        nc.vector.tensor_copy(hbf, h0f)

                        for t in range(T):
                            psR = psg.tile([128, NK, B], FP32, tag="gr",
                                           name=f"gr{t}")
                            psN = psg.tile([128, NK, B], FP32, tag="gn",
                                           name=f"gn{t}")
                            psZ = psg.tile([128, NK, B], FP32, tag="gz",
                                           name=f"gz{t}")
                            pdst = {**{m: psR[:, m, :] for m in range(4)},
                                    **{m + 4: psZ[:, m, :] for m in range(4)},
                                    **{m + 8: psN[:, m, :] for m in range(4)}}
                            for m in m_order:
                                for k in range(NK):
                                    nc.tensor.matmul(
                                        pdst[m],
                                        lhsT=whh[:, k, bass.ts(m, 128)],
                                        rhs=hbf[:, k, :],
                                        start=(k == 0), stop=(k == NK - 1))

                            hprev = h0f if t == 0 else outsT[:, :, :, t - 1]

                            rpre = gp.tile([128, NK, B], FP32, tag="rpre")
                            nc.vector.tensor_add(rpre, psR,
                                                 xgT[:, 0:4, :, t])
                            rr = gp.tile([128, NK, B], FP32, tag="rr")
                            nc.scalar.activation(rr, rpre, AF.Sigmoid)

                            hnb = gp.tile([128, NK, B], FP32, tag="hnb")
                            nc.vector.tensor_add(hnb, psN, bnw)
                            npre = gp.tile([128, NK, B], FP32, tag="npre")
                            nc.vector.tensor_mul(npre, rr, hnb)
                            nsum = gp.tile([128, NK, B], FP32, tag="nsum")
                            nc.vector.tensor_add(nsum, npre,
                                                 xgT[:, 8:12, :, t])
                            nn = gp.tile([128, NK, B], FP32, tag="nn")
                            nc.scalar.activation(nn, nsum, AF.Tanh)
                            dd = gp.tile([128, NK, B], FP32, tag="dd")
                            nc.vector.tensor_sub(dd, hprev, nn)

                            zpre = gp.tile([128, NK, B], FP32, tag="zpre")
                            nc.vector.tensor_add(zpre, psZ,
                                                 xgT[:, 4:8, :, t])
                            zz = gp.tile([128, NK, B], FP32, tag="zz")
                            nc.scalar.activation(zz, zpre, AF.Sigmoid)
                            zd = gp.tile([128, NK, B], FP32, tag="zd")
                            nc.vector.tensor_mul(zd, zz, dd)

                            hbf = hp.tile([128, NK, B], FP16, tag="hbf")
                            nc.vector.tensor_add(hbf, nn, zd)
                            nc.vector.tensor_add(outsT[:, :, :, t], nn, zd)
                            nc.scalar.activation(outs16[:, :, :, t], hbf,
                                                 AF.Copy)

                nc.sync.dma_start(d_hl.ap(), outsT[:, :, :, T - 1])

                # ------------ P4: attention, three batched passes ----------
                with tc.tile_pool(name="att", bufs=3) as ap_, \
                     tc.tile_pool(name="atall", bufs=1) as alp, \
                     tc.tile_pool(name="pss", bufs=3, space="PSUM") as pss_p, \
                     tc.tile_pool(name="pst", bufs=2, space="PSUM") as pst_p, \
                     tc.tile_pool(name="psm", bufs=2, space="PSUM") as psm_p:

                    esAll = alp.tile([64, B, S], FP32)
                    ssumAll = alp.tile([64, B], FP32)
                    attnAll = alp.tile([64, B, S], FP32)
                    recAll = alp.tile([64, B], FP32)
                    encbAll = alp.tile([128, B, H], FP16)
                    nc.scalar.dma_start(encbAll, d_enc.ap())

                    # pass 1: scores + exp/rowsum per batch (PE -> ACT)
                    for b in range(B):
                        pss = pss_p.tile([64, S], FP32, tag="sc", name=f"sc{b}")
                        for k in range(NK):
                            nc.tensor.matmul(pss,
                                             lhsT=outs16[:, k, b, :],
                                             rhs=encT[:, b, k, :],
                                             start=(k == 0), stop=(k == NK - 1))
                        nc.scalar.activation(esAll[:, b, :], pss, AF.Exp,
                                             accum_out=ssumAll[:, b:b + 1])

                    # pass 2: batched normalize (free-dim broadcast of 1/sum)
                    nc.vector.reciprocal(recAll, ssumAll)
                    rec_b = recAll.rearrange("p (b o) -> p b o", o=1).broadcast_to((64, B, S))
                    nc.vector.tensor_mul(attnAll, esAll, rec_b)
                    nc.sync.dma_start(d_attn.ap(), attnAll)

                    # pass 3: transpose + mix (PE -> DVE -> PE), pipelined
                    for b in range(B):
                        pst = pst_p.tile([128, 64], FP32, tag="tr",
                                         name=f"tr{b}")
                        nc.tensor.transpose(pst, attnAll[:, b, :],
                                            ident[0:64, 0:64])
                        atT = ap_.tile([128, 64], FP16, tag="atT")
                        nc.vector.tensor_copy(atT, pst)
                        psm = psm_p.tile([128, NK, 64], FP32, tag="mx",
                                         name=f"mx{b}")
                        for m in range(NK):
                            nc.tensor.matmul(psm[:, m, :],
                                             lhsT=encbAll[:, b, bass.ts(m, 128)],
                                             rhs=atT, start=True, stop=True)
                        nc.vector.tensor_copy(mixT[:, :, b, :], psm)

        # ------------ P5: out2T = tanh(attn_W @ combinedT + b) -----------
        with tc.tile_pool(name="pso", bufs=3, space="PSUM") as pso_p:
            for m in range(NK):
                for half in range(2):
                    pso = pso_p.tile([128, 512], FP32, tag="o2")
                    for k in range(2 * NK):
                        src = mixT if k < NK else outs16
                        rhs = src[:, k % NK, 8 * half:8 * (half + 1), :]
                        nc.tensor.matmul(pso, lhsT=awT[:, k, bass.ts(m, 128)],
                                         rhs=rhs, start=(k == 0),
                                         stop=(k == 2 * NK - 1))
                    nc.scalar.activation(
                        out2T[:, m, 8 * half:8 * (half + 1), :],
                        pso.rearrange("p (b t) -> p b t", t=T),
                        AF.Tanh, bias=biasT[:, 12 + m:13 + m])

        # ------ P6/P7: logits, exp-sum stats, AllReduce, writeout --------
        # two stages of 4 row-tiles each so stage-0 writes overlap stage-1 MMs
        with tc.tile_pool(name="lg", bufs=1, side="right") as lgp, \
             tc.tile_pool(name="esc", bufs=1, side="right") as escp, \
             tc.tile_pool(name="stat", bufs=1, side="right") as stp, \
             tc.tile_pool(name="psl", bufs=8, space="PSUM") as psl_p, \
             tc.tile_pool(name="fin", bufs=2, side="right") as finp:
            logits = lgp.tile([128, RT, VS], FP16)

            NST = 2
            RPS = RT // NST
            for stage in range(NST):
                rts = range(RPS * stage, RPS * stage + RPS)
                Scol = stp.tile([128, RPS], FP32, tag=f"sc{stage}")
                for rt in rts:
                    psl = [psl_p.tile([128, VC], FP32, tag="lg",
                                      name=f"lg{rt}_{vc_}")
                           for vc_ in range(NVC)]
                    for k in range(NK):
                        for vc in range(NVC):
                            nc.tensor.matmul(
                                psl[vc],
                                lhsT=out2T[:, k, 2 * rt:2 * rt + 2, :],
                                rhs=owT[:, k, bass.ts(vc, VC)],
                                start=(k == 0), stop=(k == NK - 1))
                    for vc in range(NVC):
                        nc.vector.tensor_add(logits[:, rt, bass.ts(vc, VC)],
                                             psl[vc], ob[:, bass.ts(vc, VC)])
                    es = escp.tile([128, VS], FP16, tag="esc")
                    nc.scalar.activation(es, logits[:, rt, :], AF.Exp,
                                         accum_out=Scol[:, rt - RPS * stage:
                                                        rt - RPS * stage + 1])

                cc_in = dramp.tile([128, RPS], FP32, tag=f"ci{stage}")
                cc_out = dramp.tile([128, RPS], FP32, tag=f"co{stage}")
                nc.sync.dma_start(cc_in, Scol)
                nc.gpsimd.collective_compute(
                    "AllReduce", ALU.add,
                    replica_groups=[list(range(NCORES))],
                    ins=[cc_in.opt()],
                    outs=[cc_out.opt()])
                Sg = stp.tile([128, RPS], FP32, tag=f"sg{stage}")
                nc.sync.dma_start(Sg, cc_out)
                lse = stp.tile([128, RPS], FP32, tag=f"ls{stage}")
                nc.scalar.activation(lse, Sg, AF.Ln)
                nlse = stp.tile([128, RPS], FP32, tag=f"nl{stage}")
                nc.vector.tensor_scalar_mul(nlse, lse, -1.0)

                for rt in rts:
                    i = rt - RPS * stage
                    ofp = finp.tile([128, VS], FP32, tag="of")
                    if rt % 4 == 3:
                        nc.scalar.activation(ofp, logits[:, rt, :],
                                             AF.Identity,
                                             bias=nlse[:, i:i + 1])
                    else:
                        nc.vector.tensor_scalar_add(ofp, logits[:, rt, :],
                                                    nlse[:, i:i + 1])
                    q = nc.sync if rt % 2 == 0 else nc.scalar
                    q.dma_start(d_lp.ap()[bass.ts(rt, 128)], ofp)


_PROGRAM = None


def _get_program():
    global _PROGRAM
    if _PROGRAM is None:
        _patch_walrus_flags()
        _PROGRAM = _build_program()
    return _PROGRAM


def _pmajor(a, nk):
    """[nk*128, X...] -> [128, nk, X...] partition-major."""
    return np.ascontiguousarray(
        a.reshape((nk, 128) + a.shape[1:]).transpose(
            (1, 0) + tuple(range(2, a.ndim + 1))))


def _prep_inputs(inputs, encoder_hidden, encoder_outputs, emb, W_ih, W_hh,
                 b_ih, b_hh, attn_W, attn_b, out_W, out_b):
    """Host-side sharding/layout: returns per-core input maps."""
    f32 = np.float32
    dec = np.asarray(inputs)[:, :-1]
    x = np.asarray(emb, f32)[dec]                       # [B, T, H]
    x2 = x.reshape(BT, H)                               # rows b*T + t
    xT = _pmajor(np.ascontiguousarray(x2.T).astype(f16), NK)

    wih = _pmajor(np.ascontiguousarray(np.asarray(W_ih, f32).T).astype(f16), NK)
    whh = _pmajor(np.ascontiguousarray(np.asarray(W_hh, f32).T).astype(f16), NK)

    b_ih = np.asarray(b_ih, f32)
    b_hh = np.asarray(b_hh, f32)
    attn_b = np.asarray(attn_b, f32)
    bias = np.zeros((128, 16), f32)
    bias[:, 0:8] = (b_ih + b_hh)[:2 * H].reshape(8, 128).T
    bias[:, 8:12] = b_ih[2 * H:].reshape(NK, 128).T
    bias[:, 12:16] = attn_b.reshape(NK, 128).T
    bnw = np.repeat(b_hh[2 * H:].reshape(NK, 128).T[:, :, None], B, axis=2)
    bnw = np.ascontiguousarray(bnw, f32)                # [128, NK, B]

    h0 = np.asarray(encoder_hidden, f32)[0]             # [B, H]
    h0T = np.ascontiguousarray(
        h0.T.reshape(NK, 128, B).transpose(1, 0, 2), f32)

    enc = np.asarray(encoder_outputs, f32)              # [B, S, H]
    encT = np.ascontiguousarray(
        enc.transpose(0, 2, 1).reshape(B, NK, 128, S).transpose(2, 0, 1, 3)
    ).astype(f16)                                       # [128, B, NK, S]
    encB = np.ascontiguousarray(enc.transpose(1, 0, 2)).astype(f16)

    awT = _pmajor(np.ascontiguousarray(np.asarray(attn_W, f32).T).astype(f16),
                  2 * NK)

    out_W = np.asarray(out_W, f32)
    out_b = np.asarray(out_b, f32)

    common = dict(xT=xT, wih=wih, whh=whh, bias=bias, bnw=bnw, h0T=h0T,
                  encT=encT, enc=encB, awT=awT)
    in_maps = []
    for c in range(NCORES):
        sl = slice(c * VS, (c + 1) * VS)
        owT = _pmajor(np.ascontiguousarray(out_W[sl].T).astype(f16), NK)
        obt = np.ascontiguousarray(
            np.broadcast_to(out_b[sl].astype(f16), (128, VS)))
        in_maps.append(dict(common, owT=owT, ob=obt))
    return in_maps


def run_raw(inputs, **run_kwargs):
    """Run the SPMD kernel; returns ((log_probs, h_last, attn), BassKernelResults)."""
    nc = _get_program()
    in_maps = _prep_inputs(**inputs)
    res = run_bass_kernel_spmd(nc, in_maps, core_ids=list(range(NCORES)),
                               **run_kwargs)
    return _assemble(res.results), res


def _assemble(outs):
    lp = np.concatenate([outs[c]["lp"] for c in range(NCORES)], axis=1)
    log_probs = lp.reshape(B, T, V).astype(np.float32)

    attn = np.ascontiguousarray(
        np.asarray(outs[0]["attn"], np.float32).transpose(1, 0, 2))

    hl = np.asarray(outs[0]["hl"], np.float32)          # [128, NK, B]
    h_last = hl.transpose(2, 1, 0).reshape(B, H)[None]  # [1, B, H]

    return log_probs, h_last, attn


def kernel(**inputs):
    out, _ = run_raw(inputs)
    return out


# revision 24
# speedup vs baseline: 1.0033x; 1.0033x over previous
"""Trainium2 Bass kernel for nn_DecoderRNN (teacher-forced GRU decoder).

Strategy (8 NeuronCores):
  - Vocab-tensor-parallel output projection: out_W/out_b sharded 4000 rows/core;
    each core computes logits[:, c*4000:(c+1)*4000] for all B*T rows plus a
    local sum(exp(logit)) per row; AllReduce(add) combines the log_softmax
    normalizer (split in two stages so the f32 writeout overlaps the second
    half's matmuls); each core writes its f32 log_prob slice.
  - GRU recurrence + attention are replicated on every core (the recurrent
    matmul is weight-load bound, so batch sharding would not speed it up, and
    every core needs all B*T hidden states for its vocab slice anyway).
  - All big matmuls run in bf16 (measured end-to-end rel-err ~3e-3), with f32
    hidden state carried between steps and f32 attention scores/softmax.

Device layout: everything "transposed dense" — feature dims on the 128 SBUF
partitions, (batch*time) along the free axis, so the per-step GRU gate math
runs full-width DVE/ACT ops of shape [128, 64]. All inputs are pre-arranged
on the host into partition-major [128, ...] blocks so every input DMA is a
single fully-contiguous per-partition read.
"""

import sys
import numpy as np
import ml_dtypes

sys.path.insert(0, "/opt/trn_rl_repo")

import concourse.bass as bass
import concourse.bacc as bacc
import concourse.mybir as mybir
import concourse.tile as tile
from concourse import bass_utils
from concourse.bass_utils import run_bass_kernel_spmd
from concourse.masks import make_identity

FP32 = mybir.dt.float32
FP16 = mybir.dt.float16
AF = mybir.ActivationFunctionType
ALU = mybir.AluOpType
AX = mybir.AxisListType

B, T, S, H, V = 16, 64, 128, 512, 32000
NCORES = 8
VS = V // NCORES           # 4000 vocab rows per core
BT = B * T                 # 1024
NK = H // 128              # 4   k-tiles of the hidden dim
NM = 3 * H // 128          # 12  m-tiles of the gate dim
RT = BT // 128             # 8   row-tiles of B*T
NVC = 8                    # vocab chunks per core
VC = VS // NVC             # 500 columns per matmul chunk

f16 = np.float16

_ENABLE_LDW_OPT = False


def _patch_walrus_flags():
    """Flip --enable-ldw-opt so LDWEIGHTS runs with fast-weight-load."""
    orig = bass_utils.run_command
    if getattr(orig, "_ldw_patched", False):
        return

    def patched(cmd, *a, **kw):
        if _ENABLE_LDW_OPT and isinstance(cmd, list):
            cmd = ["--enable-ldw-opt=true" if c == "--enable-ldw-opt=false"
                   else c for c in cmd]
        return orig(cmd, *a, **kw)

    patched._ldw_patched = True
    bass_utils.run_command = patched


def _build_program():
    nc = bacc.Bacc("TRN2", target_bir_lowering=False, debug=False,
                   num_devices=NCORES)

    d_xT = nc.dram_tensor("xT", [128, NK, BT], FP16, kind="ExternalInput")
    d_wih = nc.dram_tensor("wih", [128, NK, 3 * H], FP16, kind="ExternalInput")
    d_whh = nc.dram_tensor("whh", [128, NK, 3 * H], FP16, kind="ExternalInput")
    # bias cols: 0:8 = (b_ih+b_hh)[:1024] tiles, 8:12 = b_ih[1024:], 12:16 = attn_b
    d_bias = nc.dram_tensor("bias", [128, 16], FP32, kind="ExternalInput")
    d_bnw = nc.dram_tensor("bnw", [128, NK, B], FP32, kind="ExternalInput")
    d_h0T = nc.dram_tensor("h0T", [128, NK, B], FP32, kind="ExternalInput")
    d_encT = nc.dram_tensor("encT", [128, B, NK, S], FP16, kind="ExternalInput")
    d_enc = nc.dram_tensor("enc", [128, B, H], FP16, kind="ExternalInput")
    d_awT = nc.dram_tensor("awT", [128, 2 * NK, H], FP16, kind="ExternalInput")
    d_owT = nc.dram_tensor("owT", [128, NK, VS], FP16, kind="ExternalInput")
    d_ob = nc.dram_tensor("ob", [128, VS], FP16, kind="ExternalInput")

    d_lp = nc.dram_tensor("lp", [BT, VS], FP32, kind="ExternalOutput")
    d_attn = nc.dram_tensor("attn", [T, B, S], FP32, kind="ExternalOutput")
    d_hl = nc.dram_tensor("hl", [128, NK, B], FP32, kind="ExternalOutput")

    with tile.TileContext(nc) as tc:
        _body(tc, nc, d_xT, d_wih, d_whh, d_bias, d_bnw, d_h0T, d_encT,
              d_enc, d_awT, d_owT, d_ob, d_lp, d_attn, d_hl)

    nc.compile()
    return nc


def _body(tc, nc, d_xT, d_wih, d_whh, d_bias, d_bnw, d_h0T, d_encT, d_enc,
          d_awT, d_owT, d_ob, d_lp, d_attn, d_hl):
    # Two SBUF allocation stacks: left holds the GRU-phase tensors (freed in
    # LIFO order as phases retire), right holds the late-phase weights and
    # the tensors that survive into the logits phase.
    with tc.tile_pool(name="persist", bufs=1) as per, \
         tc.tile_pool(name="dram", bufs=1, space="DRAM") as dramp, \
         tc.tile_pool(name="bw1", bufs=1, side="right") as bw1, \
         tc.tile_pool(name="bw2", bufs=1, side="right") as bw2:

        biasT = per.tile([128, 16], FP32)
        nc.sync.dma_start(biasT, d_bias.ap())
        bnw = per.tile([128, NK, B], FP32)
        nc.sync.dma_start(bnw, d_bnw.ap())
        h0f = per.tile([128, NK, B], FP32)
        nc.sync.dma_start(h0f, d_h0T.ap())
        ident = per.tile([128, 128], FP32)
        make_identity(nc, ident)

        # tiny warmup collective: pays the cold ncfw/CC-path cost during the
        # GRU so the real normalizer AllReduces run at the warm floor
        wu_in = dramp.tile([128, 1], FP32)
        wu_out = dramp.tile([128, 1], FP32)
        wu_s = per.tile([128, 1], FP32)
        nc.any.memset(wu_s, 0.0)
        nc.sync.dma_start(wu_in, wu_s)
        nc.gpsimd.collective_compute(
            "AllReduce", ALU.add,
            replica_groups=[list(range(NCORES))],
            ins=[wu_in.opt()], outs=[wu_out.opt()])

        awT = bw1.tile([128, 2 * NK, H], FP16)
        nc.sync.dma_start(awT, d_awT.ap())
        ob = bw1.tile([128, VS], FP16)
        nc.sync.dma_start(ob, d_ob.ap())

        with tc.tile_pool(name="encp", bufs=1) as ep:
            encT = ep.tile([128, B, NK, S], FP16)
            nc.scalar.dma_start(encT, d_encT.ap())

            with tc.tile_pool(name="outs", bufs=1) as ot:
                # hidden states for all steps, transposed dense: [p, k, b, t]
                outsT = ot.tile([128, NK, B, T], FP32)

                with tc.tile_pool(name="p12", bufs=1) as p12:
                    xgT = p12.tile([128, NM, B, T], FP32)
                    whh = p12.tile([128, NK, 3 * H], FP16)

                    # ------------ P1: xgT = W_ih @ x.T (+ biases) ---------
                    with tc.tile_pool(name="gw1", bufs=1) as gw1, \
                         tc.tile_pool(name="ps1", bufs=4, space="PSUM") as ps1:
                        xT = gw1.tile([128, NK, BT], FP16)
                        wih = gw1.tile([128, NK, 3 * H], FP16)
                        for k in range(NK):
                            nc.sync.dma_start(wih[:, k, :], d_wih.ap()[:, k, :])
                            nc.sync.dma_start(xT[:, k, :], d_xT.ap()[:, k, :])
                        nc.sync.dma_start(whh, d_whh.ap())

                        for m in range(NM):
                            bcol = m if m < 8 else 8 + (m - 8)
                            pshalf = [ps1.tile([128, 512], FP32, tag="xg",
                                               name=f"xg{m}_{h_}")
                                      for h_ in range(2)]
                            for k in range(NK):
                                for half in range(2):
                                    nc.tensor.matmul(
                                        pshalf[half],
                                        lhsT=wih[:, k, bass.ts(m, 128)],
                                        rhs=xT[:, k, bass.ts(half, 512)],
                                        start=(k == 0), stop=(k == NK - 1))
                            for half in range(2):
                                dst = xgT[:, m, 8 * half:8 * (half + 1), :]
                                nc.scalar.activation(
                                    dst,
                                    pshalf[half].rearrange("p (b t) -> p b t",
                                                           t=T),
                                    AF.Identity,
                                    bias=biasT[:, bcol:bcol + 1])

                    # late-phase weights: load during the GRU
                    owT = bw2.tile([128, NK, VS], FP16)
                    nc.sync.dma_start(owT, d_owT.ap())
                    encbAll = bw2.tile([128, B, H], FP16)
                    nc.scalar.dma_start(encbAll, d_enc.ap())
                    outs16 = bw2.tile([128, NK, B, T], FP16)
                    mixT = bw2.tile([128, NK, B, T], FP16)
                    out2T = bw2.tile([128, NK, B, T], FP16)

                    # ------------ P2: GRU recurrence ----------------------
                    # emission order r(0:4), n(8:12), z(4:8); h' = n + z*(h-n)
                    m_order = [0, 1, 2, 3, 8, 9, 10, 11, 4, 5, 6, 7]
                    with tc.tile_pool(name="gru", bufs=3) as gp, \
                         tc.tile_pool(name="hb", bufs=2) as hp, \
                         tc.tile_pool(name="psg", bufs=2, space="PSUM") as psg:
                        hbf = hp.tile([128, NK, B], FP16, tag="hbf")
                        nc.vector.tensor_copy(hbf, h0f)

                        for t in range(T):
                            psR = psg.tile([128, NK, B], FP32, tag="gr",
                                           name=f"gr{t}")
                            psN = psg.tile([128, NK, B], FP32, tag="gn",
                                           name=f"gn{t}")
                            psZ = psg.tile([128, NK, B], FP32, tag="gz",
                                           name=f"gz{t}")
                            pdst = {**{m: psR[:, m, :] for m in range(4)},
                                    **{m + 4: psZ[:, m, :] for m in range(4)},
                                    **{m + 8: psN[:, m, :] for m in range(4)}}
                            for m in m_order:
                                for k in range(NK):
                                    nc.tensor.matmul(
                                        pdst[m],
                                        lhsT=whh[:, k, bass.ts(m, 128)],
                                        rhs=hbf[:, k, :],
                                        start=(k == 0), stop=(k == NK - 1))

                            hprev = h0f if t == 0 else outsT[:, :, :, t - 1]

                            rpre = gp.tile([128, NK, B], FP32, tag="rpre")
                            nc.vector.tensor_add(rpre, psR,
                                                 xgT[:, 0:4, :, t])
                            rr = gp.tile([128, NK, B], FP32, tag="rr")
                            nc.scalar.activation(rr, rpre, AF.Sigmoid)

                            hnb = gp.tile([128, NK, B], FP32, tag="hnb")
                            nc.vector.tensor_add(hnb, psN, bnw)
                            npre = gp.tile([128, NK, B], FP32, tag="npre")
                            nc.vector.tensor_mul(npre, rr, hnb)
                            nsum = gp.tile([128, NK, B], FP32, tag="nsum")
                            nc.vector.tensor_add(nsum, npre,
                                                 xgT[:, 8:12, :, t])
                            nn = gp.tile([128, NK, B], FP32, tag="nn")
                            nc.scalar.activation(nn, nsum, AF.Tanh)
                            dd = gp.tile([128, NK, B], FP32, tag="dd")
                            nc.vector.tensor_sub(dd, hprev, nn)

                            zpre = gp.tile([128, NK, B], FP32, tag="zpre")
                            nc.vector.tensor_add(zpre, psZ,
                                                 xgT[:, 4:8, :, t])
                            zz = gp.tile([128, NK, B], FP32, tag="zz")
                            nc.scalar.activation(zz, zpre, AF.Sigmoid)
                            zd = gp.tile([128, NK, B], FP32, tag="zd")
                            nc.vector.tensor_mul(zd, zz, dd)

                            hbf = hp.tile([128, NK, B], FP16, tag="hbf")
                            nc.vector.tensor_add(hbf, nn, zd)
                            nc.vector.tensor_add(outsT[:, :, :, t], nn, zd)
                            nc.scalar.activation(outs16[:, :, :, t], hbf,
                                                 AF.Copy)

                nc.sync.dma_start(d_hl.ap(), outsT[:, :, :, T - 1])

                # ------------ P4: attention, three batched passes ----------
                with tc.tile_pool(name="att", bufs=3) as ap_, \
                     tc.tile_pool(name="atall", bufs=1) as alp, \
                     tc.tile_pool(name="pss", bufs=3, space="PSUM") as pss_p, \
                     tc.tile_pool(name="pst", bufs=2, space="PSUM") as pst_p, \
                     tc.tile_pool(name="psm", bufs=2, space="PSUM") as psm_p:

                    esAll = alp.tile([64, B, S], FP32)
                    ssumAll = alp.tile([64, B], FP32)
                    attnAll = alp.tile([64, B, S], FP32)
                    recAll = alp.tile([64, B], FP32)

                    # pass 1: scores + exp/rowsum per batch (PE -> ACT)
                    for b in range(B):
                        pss = pss_p.tile([64, S], FP32, tag="sc", name=f"sc{b}")
                        for k in range(NK):
                            nc.tensor.matmul(pss,
                                             lhsT=outs16[:, k, b, :],
                                             rhs=encT[:, b, k, :],
                                             start=(k == 0), stop=(k == NK - 1))
                        nc.scalar.activation(esAll[:, b, :], pss, AF.Exp,
                                             accum_out=ssumAll[:, b:b + 1])

                    # pass 2: batched normalize (free-dim broadcast of 1/sum)
                    nc.vector.reciprocal(recAll, ssumAll)
                    rec_b = recAll.rearrange("p (b o) -> p b o", o=1).broadcast_to((64, B, S))
                    nc.vector.tensor_mul(attnAll, esAll, rec_b)
                    nc.sync.dma_start(d_attn.ap(), attnAll)

                    # pass 3: transpose + mix (PE -> DVE -> PE), pipelined
                    for b in range(B):
                        pst = pst_p.tile([128, 64], FP32, tag="tr",
                                         name=f"tr{b}")
                        nc.tensor.transpose(pst, attnAll[:, b, :],
                                            ident[0:64, 0:64])
                        atT = ap_.tile([128, 64], FP16, tag="atT")
                        nc.vector.tensor_copy(atT, pst)
                        psm = psm_p.tile([128, NK, 64], FP32, tag="mx",
                                         name=f"mx{b}")
                        for m in range(NK):
                            nc.tensor.matmul(psm[:, m, :],
                                             lhsT=encbAll[:, b, bass.ts(m, 128)],
                                             rhs=atT, start=True, stop=True)
                        nc.vector.tensor_copy(mixT[:, :, b, :], psm)

        # ------------ P5: out2T = tanh(attn_W @ combinedT + b) -----------
        with tc.tile_pool(name="pso", bufs=3, space="PSUM") as pso_p:
            for m in range(NK):
                for half in range(2):
                    pso = pso_p.tile([128, 512], FP32, tag="o2")
                    for k in range(2 * NK):
                        src = mixT if k < NK else outs16
                        rhs = src[:, k % NK, 8 * half:8 * (half + 1), :]
                        nc.tensor.matmul(pso, lhsT=awT[:, k, bass.ts(m, 128)],
                                         rhs=rhs, start=(k == 0),
                                         stop=(k == 2 * NK - 1))
                    nc.scalar.activation(
                        out2T[:, m, 8 * half:8 * (half + 1), :],
                        pso.rearrange("p (b t) -> p b t", t=T),
                        AF.Tanh, bias=biasT[:, 12 + m:13 + m])

        # ------ P6/P7: logits, exp-sum stats, AllReduce, writeout --------
        # two stages of 4 row-tiles each so stage-0 writes overlap stage-1 MMs
        with tc.tile_pool(name="lg", bufs=1, side="right") as lgp, \
             tc.tile_pool(name="esc", bufs=1, side="right") as escp, \
             tc.tile_pool(name="stat", bufs=1, side="right") as stp, \
             tc.tile_pool(name="psl", bufs=8, space="PSUM") as psl_p, \
             tc.tile_pool(name="fin", bufs=2, side="right") as finp:
            logits = lgp.tile([128, RT, VS], FP16)

            NST = 2
            RPS = RT // NST
            for stage in range(NST):
                rts = range(RPS * stage, RPS * stage + RPS)
                Scol = stp.tile([128, RPS], FP32, tag=f"sc{stage}")
                for rt in rts:
                    psl = [psl_p.tile([128, VC], FP32, tag="lg",
                                      name=f"lg{rt}_{vc_}")
                           for vc_ in range(NVC)]
                    for k in range(NK):
                        for vc in range(NVC):
                            nc.tensor.matmul(
                                psl[vc],
                                lhsT=out2T[:, k, 2 * rt:2 * rt + 2, :],
                                rhs=owT[:, k, bass.ts(vc, VC)],
                                start=(k == 0), stop=(k == NK - 1))
                    for vc in range(NVC):
                        nc.vector.tensor_add(logits[:, rt, bass.ts(vc, VC)],
                                             psl[vc], ob[:, bass.ts(vc, VC)])
                    es = escp.tile([128, VS], FP16, tag="esc")
                    nc.scalar.activation(es, logits[:, rt, :], AF.Exp,
                                         accum_out=Scol[:, rt - RPS * stage:
                                                        rt - RPS * stage + 1])

                cc_in = dramp.tile([128, RPS], FP32, tag=f"ci{stage}")
                cc_out = dramp.tile([128, RPS], FP32, tag=f"co{stage}")
                nc.sync.dma_start(cc_in, Scol)
                nc.gpsimd.collective_compute(
                    "AllReduce", ALU.add,
                    replica_groups=[list(range(NCORES))],
                    ins=[cc_in.opt()],
                    outs=[cc_out.opt()])
                Sg = stp.tile([128, RPS], FP32, tag=f"sg{stage}")
                nc.sync.dma_start(Sg, cc_out)
                lse = stp.tile([128, RPS], FP32, tag=f"ls{stage}")
                nc.scalar.activation(lse, Sg, AF.Ln)
                nlse = stp.tile([128, RPS], FP32, tag=f"nl{stage}")
                nc.vector.tensor_scalar_mul(nlse, lse, -1.0)

                for rt in rts:
                    i = rt - RPS * stage
                    ofp = finp.tile([128, VS], FP32, tag="of")
                    if rt % 4 == 3:
                        nc.scalar.activation(ofp, logits[:, rt, :],
                                             AF.Identity,
                                             bias=nlse[:, i:i + 1])
                    else:
                        nc.vector.tensor_scalar_add(ofp, logits[:, rt, :],
                                                    nlse[:, i:i + 1])
                    q = nc.sync if rt % 2 == 0 else nc.scalar
                    q.dma_start(d_lp.ap()[bass.ts(rt, 128)], ofp)


_PROGRAM = None


def _get_program():
    global _PROGRAM
    if _PROGRAM is None:
        _patch_walrus_flags()
        _PROGRAM = _build_program()
    return _PROGRAM


def _pmajor(a, nk):
    """[nk*128, X...] -> [128, nk, X...] partition-major."""
    return np.ascontiguousarray(
        a.reshape((nk, 128) + a.shape[1:]).transpose(
            (1, 0) + tuple(range(2, a.ndim + 1))))


def _prep_inputs(inputs, encoder_hidden, encoder_outputs, emb, W_ih, W_hh,
                 b_ih, b_hh, attn_W, attn_b, out_W, out_b):
    """Host-side sharding/layout: returns per-core input maps."""
    f32 = np.float32
    dec = np.asarray(inputs)[:, :-1]
    x = np.asarray(emb, f32)[dec]                       # [B, T, H]
    x2 = x.reshape(BT, H)                               # rows b*T + t
    xT = _pmajor(np.ascontiguousarray(x2.T).astype(f16), NK)

    wih = _pmajor(np.ascontiguousarray(np.asarray(W_ih, f32).T).astype(f16), NK)
    whh = _pmajor(np.ascontiguousarray(np.asarray(W_hh, f32).T).astype(f16), NK)

    b_ih = np.asarray(b_ih, f32)
    b_hh = np.asarray(b_hh, f32)
    attn_b = np.asarray(attn_b, f32)
    bias = np.zeros((128, 16), f32)
    bias[:, 0:8] = (b_ih + b_hh)[:2 * H].reshape(8, 128).T
    bias[:, 8:12] = b_ih[2 * H:].reshape(NK, 128).T
    bias[:, 12:16] = attn_b.reshape(NK, 128).T
    bnw = np.repeat(b_hh[2 * H:].reshape(NK, 128).T[:, :, None], B, axis=2)
    bnw = np.ascontiguousarray(bnw, f32)                # [128, NK, B]

    h0 = np.asarray(encoder_hidden, f32)[0]             # [B, H]
    h0T = np.ascontiguousarray(
        h0.T.reshape(NK, 128, B).transpose(1, 0, 2), f32)

    enc = np.asarray(encoder_outputs, f32)              # [B, S, H]
    encT = np.ascontiguousarray(
        enc.transpose(0, 2, 1).reshape(B, NK, 128, S).transpose(2, 0, 1, 3)
    ).astype(f16)                                       # [128, B, NK, S]
    encB = np.ascontiguousarray(enc.transpose(1, 0, 2)).astype(f16)

    awT = _pmajor(np.ascontiguousarray(np.asarray(attn_W, f32).T).astype(f16),
                  2 * NK)

    out_W = np.asarray(out_W, f32)
    out_b = np.asarray(out_b, f32)

    common = dict(xT=xT, wih=wih, whh=whh, bias=bias, bnw=bnw, h0T=h0T,
                  encT=encT, enc=encB, awT=awT)
    in_maps = []
    for c in range(NCORES):
        sl = slice(c * VS, (c + 1) * VS)
        owT = _pmajor(np.ascontiguousarray(out_W[sl].T).astype(f16), NK)
        obt = np.ascontiguousarray(
            np.broadcast_to(out_b[sl].astype(f16), (128, VS)))
        in_maps.append(dict(common, owT=owT, ob=obt))
    return in_maps


def run_raw(inputs, **run_kwargs):
    """Run the SPMD kernel; returns ((log_probs, h_last, attn), BassKernelResults)."""
    nc = _get_program()
    in_maps = _prep_inputs(**inputs)
    res = run_bass_kernel_spmd(nc, in_maps, core_ids=list(range(NCORES)),
                               **run_kwargs)
    return _assemble(res.results), res


def _assemble(outs):
    lp = np.concatenate([outs[c]["lp"] for c in range(NCORES)], axis=1)
    log_probs = lp.reshape(B, T, V).astype(np.float32)

    attn = np.ascontiguousarray(
        np.asarray(outs[0]["attn"], np.float32).transpose(1, 0, 2))

    hl = np.asarray(outs[0]["hl"], np.float32)          # [128, NK, B]
    h_last = hl.transpose(2, 1, 0).reshape(B, H)[None]  # [1, B, H]

    return log_probs, h_last, attn


def kernel(**inputs):
    out, _ = run_raw(inputs)
    return out


# revision 25
# speedup vs baseline: 1.0209x; 1.0175x over previous
"""Trainium2 Bass kernel for nn_DecoderRNN (teacher-forced GRU decoder).

Strategy (8 NeuronCores):
  - Vocab-tensor-parallel output projection: out_W/out_b sharded 4000 rows/core;
    each core computes logits[:, c*4000:(c+1)*4000] for all B*T rows plus a
    local sum(exp(logit)) per row; AllReduce(add) combines the log_softmax
    normalizer (split in two stages so the f32 writeout overlaps the second
    half's matmuls); each core writes its f32 log_prob slice.
  - GRU recurrence + attention are replicated on every core (the recurrent
    matmul is weight-load bound, so batch sharding would not speed it up, and
    every core needs all B*T hidden states for its vocab slice anyway).
  - All big matmuls run in bf16 (measured end-to-end rel-err ~3e-3), with f32
    hidden state carried between steps and f32 attention scores/softmax.

Device layout: everything "transposed dense" — feature dims on the 128 SBUF
partitions, (batch*time) along the free axis, so the per-step GRU gate math
runs full-width DVE/ACT ops of shape [128, 64]. All inputs are pre-arranged
on the host into partition-major [128, ...] blocks so every input DMA is a
single fully-contiguous per-partition read.
"""

import sys
import numpy as np
import ml_dtypes

sys.path.insert(0, "/opt/trn_rl_repo")

import concourse.bass as bass
import concourse.bacc as bacc
import concourse.mybir as mybir
import concourse.tile as tile
from concourse import bass_utils
from concourse.bass_utils import run_bass_kernel_spmd
from concourse.masks import make_identity

FP32 = mybir.dt.float32
FP16 = mybir.dt.float16
AF = mybir.ActivationFunctionType
ALU = mybir.AluOpType
AX = mybir.AxisListType

B, T, S, H, V = 16, 64, 128, 512, 32000
NCORES = 8
VS = V // NCORES           # 4000 vocab rows per core
BT = B * T                 # 1024
NK = H // 128              # 4   k-tiles of the hidden dim
NM = 3 * H // 128          # 12  m-tiles of the gate dim
RT = BT // 128             # 8   row-tiles of B*T
NVC = 8                    # vocab chunks per core
VC = VS // NVC             # 500 columns per matmul chunk

f16 = np.float16

_ENABLE_LDW_OPT = False


def _patch_walrus_flags():
    """Flip --enable-ldw-opt so LDWEIGHTS runs with fast-weight-load."""
    orig = bass_utils.run_command
    if getattr(orig, "_ldw_patched", False):
        return

    def patched(cmd, *a, **kw):
        if _ENABLE_LDW_OPT and isinstance(cmd, list):
            cmd = ["--enable-ldw-opt=true" if c == "--enable-ldw-opt=false"
                   else c for c in cmd]
        return orig(cmd, *a, **kw)

    patched._ldw_patched = True
    bass_utils.run_command = patched


def _build_program():
    nc = bacc.Bacc("TRN2", target_bir_lowering=False, debug=False,
                   num_devices=NCORES)

    d_xT = nc.dram_tensor("xT", [128, NK, BT], FP16, kind="ExternalInput")
    d_wih = nc.dram_tensor("wih", [128, NK, 3 * H], FP16, kind="ExternalInput")
    d_whh = nc.dram_tensor("whh", [128, NK, 3 * H], FP16, kind="ExternalInput")
    # bias cols: 0:8 = (b_ih+b_hh)[:1024] tiles, 8:12 = b_ih[1024:], 12:16 = attn_b
    d_bias = nc.dram_tensor("bias", [128, 16], FP32, kind="ExternalInput")
    d_bnw = nc.dram_tensor("bnw", [128, NK, B], FP32, kind="ExternalInput")
    d_h0T = nc.dram_tensor("h0T", [128, NK, B], FP32, kind="ExternalInput")
    d_encT = nc.dram_tensor("encT", [128, B, NK, S], FP16, kind="ExternalInput")
    d_enc = nc.dram_tensor("enc", [128, B, H], FP16, kind="ExternalInput")
    d_awT = nc.dram_tensor("awT", [128, 2 * NK, H], FP16, kind="ExternalInput")
    d_owT = nc.dram_tensor("owT", [128, NK, VS], FP16, kind="ExternalInput")
    d_ob = nc.dram_tensor("ob", [128, VS], FP16, kind="ExternalInput")

    d_lp = nc.dram_tensor("lp", [BT, VS], FP32, kind="ExternalOutput")
    d_attn = nc.dram_tensor("attn", [T, B, S], FP32, kind="ExternalOutput")
    d_hl = nc.dram_tensor("hl", [128, NK, B], FP32, kind="ExternalOutput")

    with tile.TileContext(nc) as tc:
        _body(tc, nc, d_xT, d_wih, d_whh, d_bias, d_bnw, d_h0T, d_encT,
              d_enc, d_awT, d_owT, d_ob, d_lp, d_attn, d_hl)

    nc.compile()
    return nc


def _body(tc, nc, d_xT, d_wih, d_whh, d_bias, d_bnw, d_h0T, d_encT, d_enc,
          d_awT, d_owT, d_ob, d_lp, d_attn, d_hl):
    # Two SBUF allocation stacks: left holds the GRU-phase tensors (freed in
    # LIFO order as phases retire), right holds the late-phase weights and
    # the tensors that survive into the logits phase.
    with tc.tile_pool(name="persist", bufs=1) as per, \
         tc.tile_pool(name="dram", bufs=1, space="DRAM") as dramp, \
         tc.tile_pool(name="bw1", bufs=1, side="right") as bw1, \
         tc.tile_pool(name="bw2", bufs=1, side="right") as bw2:

        biasT = per.tile([128, 16], FP32)
        nc.sync.dma_start(biasT, d_bias.ap())
        bnw = per.tile([128, NK, B], FP32)
        nc.sync.dma_start(bnw, d_bnw.ap())
        h0f = per.tile([128, NK, B], FP32)
        nc.sync.dma_start(h0f, d_h0T.ap())
        ident = per.tile([128, 128], FP32)
        make_identity(nc, ident)

        # tiny warmup collective: pays the cold ncfw/CC-path cost during the
        # GRU so the real normalizer AllReduces run at the warm floor
        wu_in = dramp.tile([128, 1], FP32)
        wu_out = dramp.tile([128, 1], FP32)
        wu_s = per.tile([128, 1], FP32)
        nc.any.memset(wu_s, 0.0)
        nc.gpsimd.dma_start(wu_in, wu_s)
        nc.gpsimd.collective_compute(
            "AllReduce", ALU.add,
            replica_groups=[list(range(NCORES))],
            ins=[wu_in.opt()], outs=[wu_out.opt()])

        awT = bw1.tile([128, 2 * NK, H], FP16)
        nc.sync.dma_start(awT, d_awT.ap())
        ob = bw1.tile([128, VS], FP16)
        nc.sync.dma_start(ob, d_ob.ap())

        with tc.tile_pool(name="encp", bufs=1) as ep:
            encT = ep.tile([128, B, NK, S], FP16)
            nc.scalar.dma_start(encT, d_encT.ap())

            with tc.tile_pool(name="outs", bufs=1) as ot:
                # hidden states for all steps, transposed dense: [p, k, b, t]
                outsT = ot.tile([128, NK, B, T], FP32)

                with tc.tile_pool(name="p12", bufs=1) as p12:
                    xgT = p12.tile([128, NM, B, T], FP32)
                    whh = p12.tile([128, NK, 3 * H], FP16)

                    # ------------ P1: xgT = W_ih @ x.T (+ biases) ---------
                    with tc.tile_pool(name="gw1", bufs=1) as gw1, \
                         tc.tile_pool(name="ps1", bufs=4, space="PSUM") as ps1:
                        xT = gw1.tile([128, NK, BT], FP16)
                        wih = gw1.tile([128, NK, 3 * H], FP16)
                        for k in range(NK):
                            nc.sync.dma_start(wih[:, k, :], d_wih.ap()[:, k, :])
                            nc.sync.dma_start(xT[:, k, :], d_xT.ap()[:, k, :])
                        nc.sync.dma_start(whh, d_whh.ap())

                        for m in range(NM):
                            bcol = m if m < 8 else 8 + (m - 8)
                            pshalf = [ps1.tile([128, 512], FP32, tag="xg",
                                               name=f"xg{m}_{h_}")
                                      for h_ in range(2)]
                            for k in range(NK):
                                for half in range(2):
                                    nc.tensor.matmul(
                                        pshalf[half],
                                        lhsT=wih[:, k, bass.ts(m, 128)],
                                        rhs=xT[:, k, bass.ts(half, 512)],
                                        start=(k == 0), stop=(k == NK - 1))
                            for half in range(2):
                                dst = xgT[:, m, 8 * half:8 * (half + 1), :]
                                nc.scalar.activation(
                                    dst,
                                    pshalf[half].rearrange("p (b t) -> p b t",
                                                           t=T),
                                    AF.Identity,
                                    bias=biasT[:, bcol:bcol + 1])

                    # late-phase weights: load during the GRU
                    owT = bw2.tile([128, NK, VS], FP16)
                    nc.sync.dma_start(owT, d_owT.ap())
                    encbAll = bw2.tile([128, B, H], FP16)
                    nc.scalar.dma_start(encbAll, d_enc.ap())
                    outs16 = bw2.tile([128, NK, B, T], FP16)
                    mixT = bw2.tile([128, NK, B, T], FP16)
                    out2T = bw2.tile([128, NK, B, T], FP16)

                    # ------------ P2: GRU recurrence ----------------------
                    # emission order r(0:4), n(8:12), z(4:8); h' = n + z*(h-n)
                    m_order = [0, 1, 2, 3, 8, 9, 10, 11, 4, 5, 6, 7]
                    with tc.tile_pool(name="gru", bufs=3) as gp, \
                         tc.tile_pool(name="hb", bufs=2) as hp, \
                         tc.tile_pool(name="psg", bufs=2, space="PSUM") as psg:
                        hbf = hp.tile([128, NK, B], FP16, tag="hbf")
                        nc.vector.tensor_copy(hbf, h0f)

                        for t in range(T):
                            psR = psg.tile([128, NK, B], FP32, tag="gr",
                                           name=f"gr{t}")
                            psN = psg.tile([128, NK, B], FP32, tag="gn",
                                           name=f"gn{t}")
                            psZ = psg.tile([128, NK, B], FP32, tag="gz",
                                           name=f"gz{t}")
                            pdst = {**{m: psR[:, m, :] for m in range(4)},
                                    **{m + 4: psZ[:, m, :] for m in range(4)},
                                    **{m + 8: psN[:, m, :] for m in range(4)}}
                            for m in m_order:
                                for k in range(NK):
                                    nc.tensor.matmul(
                                        pdst[m],
                                        lhsT=whh[:, k, bass.ts(m, 128)],
                                        rhs=hbf[:, k, :],
                                        start=(k == 0), stop=(k == NK - 1))

                            hprev = h0f if t == 0 else outsT[:, :, :, t - 1]

                            rpre = gp.tile([128, NK, B], FP32, tag="rpre")
                            nc.vector.tensor_add(rpre, psR,
                                                 xgT[:, 0:4, :, t])
                            rr = gp.tile([128, NK, B], FP32, tag="rr")
                            nc.scalar.activation(rr, rpre, AF.Sigmoid)

                            hnb = gp.tile([128, NK, B], FP32, tag="hnb")
                            nc.vector.tensor_add(hnb, psN, bnw)
                            npre = gp.tile([128, NK, B], FP32, tag="npre")
                            nc.vector.tensor_mul(npre, rr, hnb)
                            nsum = gp.tile([128, NK, B], FP32, tag="nsum")
                            nc.vector.tensor_add(nsum, npre,
                                                 xgT[:, 8:12, :, t])
                            nn = gp.tile([128, NK, B], FP32, tag="nn")
                            nc.scalar.activation(nn, nsum, AF.Tanh)
                            dd = gp.tile([128, NK, B], FP32, tag="dd")
                            nc.vector.tensor_sub(dd, hprev, nn)

                            zpre = gp.tile([128, NK, B], FP32, tag="zpre")
                            nc.vector.tensor_add(zpre, psZ,
                                                 xgT[:, 4:8, :, t])
                            zz = gp.tile([128, NK, B], FP32, tag="zz")
                            nc.scalar.activation(zz, zpre, AF.Sigmoid)
                            zd = gp.tile([128, NK, B], FP32, tag="zd")
                            nc.vector.tensor_mul(zd, zz, dd)

                            hbf = hp.tile([128, NK, B], FP16, tag="hbf")
                            nc.vector.tensor_add(hbf, nn, zd)
                            nc.vector.tensor_add(outsT[:, :, :, t], nn, zd)
                            nc.scalar.activation(outs16[:, :, :, t], hbf,
                                                 AF.Copy)

                nc.sync.dma_start(d_hl.ap(), outsT[:, :, :, T - 1])

                # ------------ P4: attention, three batched passes ----------
                with tc.tile_pool(name="att", bufs=3) as ap_, \
                     tc.tile_pool(name="atall", bufs=1) as alp, \
                     tc.tile_pool(name="pss", bufs=3, space="PSUM") as pss_p, \
                     tc.tile_pool(name="pst", bufs=2, space="PSUM") as pst_p, \
                     tc.tile_pool(name="psm", bufs=2, space="PSUM") as psm_p:

                    esAll = alp.tile([64, B, S], FP32)
                    ssumAll = alp.tile([64, B], FP32)
                    attnAll = alp.tile([64, B, S], FP32)
                    recAll = alp.tile([64, B], FP32)

                    # pass 1: scores + exp/rowsum per batch (PE -> ACT)
                    for b in range(B):
                        pss = pss_p.tile([64, S], FP32, tag="sc", name=f"sc{b}")
                        for k in range(NK):
                            nc.tensor.matmul(pss,
                                             lhsT=outs16[:, k, b, :],
                                             rhs=encT[:, b, k, :],
                                             start=(k == 0), stop=(k == NK - 1))
                        nc.scalar.activation(esAll[:, b, :], pss, AF.Exp,
                                             accum_out=ssumAll[:, b:b + 1])

                    # pass 2: batched normalize (free-dim broadcast of 1/sum)
                    nc.vector.reciprocal(recAll, ssumAll)
                    rec_b = recAll.rearrange("p (b o) -> p b o", o=1).broadcast_to((64, B, S))
                    nc.vector.tensor_mul(attnAll, esAll, rec_b)
                    nc.sync.dma_start(d_attn.ap(), attnAll)

                    # pass 3: transpose + mix (PE -> DVE -> PE), pipelined
                    for b in range(B):
                        pst = pst_p.tile([128, 64], FP32, tag="tr",
                                         name=f"tr{b}")
                        nc.tensor.transpose(pst, attnAll[:, b, :],
                                            ident[0:64, 0:64])
                        atT = ap_.tile([128, 64], FP16, tag="atT")
                        nc.vector.tensor_copy(atT, pst)
                        psm = psm_p.tile([128, NK, 64], FP32, tag="mx",
                                         name=f"mx{b}")
                        for m in range(NK):
                            nc.tensor.matmul(psm[:, m, :],
                                             lhsT=encbAll[:, b, bass.ts(m, 128)],
                                             rhs=atT, start=True, stop=True)
                        nc.vector.tensor_copy(mixT[:, :, b, :], psm)

        # ------------ P5: out2T = tanh(attn_W @ combinedT + b) -----------
        with tc.tile_pool(name="pso", bufs=3, space="PSUM") as pso_p:
            for m in range(NK):
                for half in range(2):
                    pso = pso_p.tile([128, 512], FP32, tag="o2")
                    for k in range(2 * NK):
                        src = mixT if k < NK else outs16
                        rhs = src[:, k % NK, 8 * half:8 * (half + 1), :]
                        nc.tensor.matmul(pso, lhsT=awT[:, k, bass.ts(m, 128)],
                                         rhs=rhs, start=(k == 0),
                                         stop=(k == 2 * NK - 1))
                    nc.scalar.activation(
                        out2T[:, m, 8 * half:8 * (half + 1), :],
                        pso.rearrange("p (b t) -> p b t", t=T),
                        AF.Tanh, bias=biasT[:, 12 + m:13 + m])

        # ------ P6/P7: logits, exp-sum stats, AllReduce, writeout --------
        # two stages of 4 row-tiles each so stage-0 writes overlap stage-1 MMs
        with tc.tile_pool(name="lg", bufs=1, side="right") as lgp, \
             tc.tile_pool(name="esc", bufs=1, side="right") as escp, \
             tc.tile_pool(name="stat", bufs=1, side="right") as stp, \
             tc.tile_pool(name="psl", bufs=8, space="PSUM") as psl_p, \
             tc.tile_pool(name="fin", bufs=2, side="right") as finp:
            logits = lgp.tile([128, RT, VS], FP16)

            NST = 2
            RPS = RT // NST
            for stage in range(NST):
                rts = range(RPS * stage, RPS * stage + RPS)
                Scol = stp.tile([128, RPS], FP32, tag=f"sc{stage}")
                for rt in rts:
                    psl = [psl_p.tile([128, VC], FP32, tag="lg",
                                      name=f"lg{rt}_{vc_}")
                           for vc_ in range(NVC)]
                    for k in range(NK):
                        for vc in range(NVC):
                            nc.tensor.matmul(
                                psl[vc],
                                lhsT=out2T[:, k, 2 * rt:2 * rt + 2, :],
                                rhs=owT[:, k, bass.ts(vc, VC)],
                                start=(k == 0), stop=(k == NK - 1))
                    for vc in range(NVC):
                        nc.vector.tensor_add(logits[:, rt, bass.ts(vc, VC)],
                                             psl[vc], ob[:, bass.ts(vc, VC)])
                    es = escp.tile([128, VS], FP16, tag="esc")
                    nc.scalar.activation(es, logits[:, rt, :], AF.Exp,
                                         accum_out=Scol[:, rt - RPS * stage:
                                                        rt - RPS * stage + 1])

                cc_in = dramp.tile([128, RPS], FP32, tag=f"ci{stage}")
                cc_out = dramp.tile([128, RPS], FP32, tag=f"co{stage}")
                nc.gpsimd.dma_start(cc_in, Scol)
                nc.gpsimd.collective_compute(
                    "AllReduce", ALU.add,
                    replica_groups=[list(range(NCORES))],
                    ins=[cc_in.opt()],
                    outs=[cc_out.opt()])
                Sg = stp.tile([128, RPS], FP32, tag=f"sg{stage}")
                nc.gpsimd.dma_start(Sg, cc_out)
                lse = stp.tile([128, RPS], FP32, tag=f"ls{stage}")
                nc.scalar.activation(lse, Sg, AF.Ln)
                nlse = stp.tile([128, RPS], FP32, tag=f"nl{stage}")
                nc.vector.tensor_scalar_mul(nlse, lse, -1.0)

                for rt in rts:
                    i = rt - RPS * stage
                    ofp = finp.tile([128, VS], FP32, tag="of")
                    if rt % 4 == 3:
                        nc.scalar.activation(ofp, logits[:, rt, :],
                                             AF.Identity,
                                             bias=nlse[:, i:i + 1])
                    else:
                        nc.vector.tensor_scalar_add(ofp, logits[:, rt, :],
                                                    nlse[:, i:i + 1])
                    q = nc.sync if rt % 2 == 0 else nc.scalar
                    q.dma_start(d_lp.ap()[bass.ts(rt, 128)], ofp)


_PROGRAM = None


def _get_program():
    global _PROGRAM
    if _PROGRAM is None:
        _patch_walrus_flags()
        _PROGRAM = _build_program()
    return _PROGRAM


def _pmajor(a, nk):
    """[nk*128, X...] -> [128, nk, X...] partition-major."""
    return np.ascontiguousarray(
        a.reshape((nk, 128) + a.shape[1:]).transpose(
            (1, 0) + tuple(range(2, a.ndim + 1))))


def _prep_inputs(inputs, encoder_hidden, encoder_outputs, emb, W_ih, W_hh,
                 b_ih, b_hh, attn_W, attn_b, out_W, out_b):
    """Host-side sharding/layout: returns per-core input maps."""
    f32 = np.float32
    dec = np.asarray(inputs)[:, :-1]
    x = np.asarray(emb, f32)[dec]                       # [B, T, H]
    x2 = x.reshape(BT, H)                               # rows b*T + t
    xT = _pmajor(np.ascontiguousarray(x2.T).astype(f16), NK)

    wih = _pmajor(np.ascontiguousarray(np.asarray(W_ih, f32).T).astype(f16), NK)
    whh = _pmajor(np.ascontiguousarray(np.asarray(W_hh, f32).T).astype(f16), NK)

    b_ih = np.asarray(b_ih, f32)
    b_hh = np.asarray(b_hh, f32)
    attn_b = np.asarray(attn_b, f32)
    bias = np.zeros((128, 16), f32)
    bias[:, 0:8] = (b_ih + b_hh)[:2 * H].reshape(8, 128).T
    bias[:, 8:12] = b_ih[2 * H:].reshape(NK, 128).T
    bias[:, 12:16] = attn_b.reshape(NK, 128).T
    bnw = np.repeat(b_hh[2 * H:].reshape(NK, 128).T[:, :, None], B, axis=2)
    bnw = np.ascontiguousarray(bnw, f32)                # [128, NK, B]

    h0 = np.asarray(encoder_hidden, f32)[0]             # [B, H]
    h0T = np.ascontiguousarray(
        h0.T.reshape(NK, 128, B).transpose(1, 0, 2), f32)

    enc = np.asarray(encoder_outputs, f32)              # [B, S, H]
    encT = np.ascontiguousarray(
        enc.transpose(0, 2, 1).reshape(B, NK, 128, S).transpose(2, 0, 1, 3)
    ).astype(f16)                                       # [128, B, NK, S]
    encB = np.ascontiguousarray(enc.transpose(1, 0, 2)).astype(f16)

    awT = _pmajor(np.ascontiguousarray(np.asarray(attn_W, f32).T).astype(f16),
                  2 * NK)

    out_W = np.asarray(out_W, f32)
    out_b = np.asarray(out_b, f32)

    common = dict(xT=xT, wih=wih, whh=whh, bias=bias, bnw=bnw, h0T=h0T,
                  encT=encT, enc=encB, awT=awT)
    in_maps = []
    for c in range(NCORES):
        sl = slice(c * VS, (c + 1) * VS)
        owT = _pmajor(np.ascontiguousarray(out_W[sl].T).astype(f16), NK)
        obt = np.ascontiguousarray(
            np.broadcast_to(out_b[sl].astype(f16), (128, VS)))
        in_maps.append(dict(common, owT=owT, ob=obt))
    return in_maps


def run_raw(inputs, **run_kwargs):
    """Run the SPMD kernel; returns ((log_probs, h_last, attn), BassKernelResults)."""
    nc = _get_program()
    in_maps = _prep_inputs(**inputs)
    res = run_bass_kernel_spmd(nc, in_maps, core_ids=list(range(NCORES)),
                               **run_kwargs)
    return _assemble(res.results), res


def _assemble(outs):
    lp = np.concatenate([outs[c]["lp"] for c in range(NCORES)], axis=1)
    log_probs = lp.reshape(B, T, V).astype(np.float32)

    attn = np.ascontiguousarray(
        np.asarray(outs[0]["attn"], np.float32).transpose(1, 0, 2))

    hl = np.asarray(outs[0]["hl"], np.float32)          # [128, NK, B]
    h_last = hl.transpose(2, 1, 0).reshape(B, H)[None]  # [1, B, H]

    return log_probs, h_last, attn


def kernel(**inputs):
    out, _ = run_raw(inputs)
    return out


# revision 26
# speedup vs baseline: 1.0284x; 1.0073x over previous
"""Trainium2 Bass kernel for nn_DecoderRNN (teacher-forced GRU decoder).

Strategy (8 NeuronCores):
  - Vocab-tensor-parallel output projection: out_W/out_b sharded 4000 rows/core;
    each core computes logits[:, c*4000:(c+1)*4000] for all B*T rows plus a
    local sum(exp(logit)) per row; AllReduce(add) combines the log_softmax
    normalizer (split in two stages so the f32 writeout overlaps the second
    half's matmuls); each core writes its f32 log_prob slice.
  - GRU recurrence + attention are replicated on every core (the recurrent
    matmul is weight-load bound, so batch sharding would not speed it up, and
    every core needs all B*T hidden states for its vocab slice anyway).
  - All big matmuls run in bf16 (measured end-to-end rel-err ~3e-3), with f32
    hidden state carried between steps and f32 attention scores/softmax.

Device layout: everything "transposed dense" — feature dims on the 128 SBUF
partitions, (batch*time) along the free axis, so the per-step GRU gate math
runs full-width DVE/ACT ops of shape [128, 64]. All inputs are pre-arranged
on the host into partition-major [128, ...] blocks so every input DMA is a
single fully-contiguous per-partition read.
"""

import sys
import numpy as np
import ml_dtypes

sys.path.insert(0, "/opt/trn_rl_repo")

import concourse.bass as bass
import concourse.bacc as bacc
import concourse.mybir as mybir
import concourse.tile as tile
from concourse import bass_utils
from concourse.bass_utils import run_bass_kernel_spmd
from concourse.masks import make_identity

FP32 = mybir.dt.float32
FP16 = mybir.dt.float16
AF = mybir.ActivationFunctionType
ALU = mybir.AluOpType
AX = mybir.AxisListType

B, T, S, H, V = 16, 64, 128, 512, 32000
NCORES = 8
VS = V // NCORES           # 4000 vocab rows per core
BT = B * T                 # 1024
NK = H // 128              # 4   k-tiles of the hidden dim
NM = 3 * H // 128          # 12  m-tiles of the gate dim
RT = BT // 128             # 8   row-tiles of B*T
NVC = 8                    # vocab chunks per core
VC = VS // NVC             # 500 columns per matmul chunk

f16 = np.float16

_ENABLE_LDW_OPT = False


def _patch_walrus_flags():
    """Flip --enable-ldw-opt so LDWEIGHTS runs with fast-weight-load."""
    orig = bass_utils.run_command
    if getattr(orig, "_ldw_patched", False):
        return

    def patched(cmd, *a, **kw):
        if _ENABLE_LDW_OPT and isinstance(cmd, list):
            cmd = ["--enable-ldw-opt=true" if c == "--enable-ldw-opt=false"
                   else c for c in cmd]
        return orig(cmd, *a, **kw)

    patched._ldw_patched = True
    bass_utils.run_command = patched


def _build_program():
    nc = bacc.Bacc("TRN2", target_bir_lowering=False, debug=False,
                   num_devices=NCORES)

    d_xT = nc.dram_tensor("xT", [128, NK, BT], FP16, kind="ExternalInput")
    d_wih = nc.dram_tensor("wih", [128, NK, 3 * H], FP16, kind="ExternalInput")
    d_whh = nc.dram_tensor("whh", [128, NK, 3 * H], FP16, kind="ExternalInput")
    # bias cols: 0:8 = (b_ih+b_hh)[:1024] tiles, 8:12 = b_ih[1024:], 12:16 = attn_b
    d_bias = nc.dram_tensor("bias", [128, 16], FP32, kind="ExternalInput")
    d_bnw = nc.dram_tensor("bnw", [128, NK, B], FP32, kind="ExternalInput")
    d_h0T = nc.dram_tensor("h0T", [128, NK, B], FP32, kind="ExternalInput")
    d_encT = nc.dram_tensor("encT", [128, B, NK, S], FP16, kind="ExternalInput")
    d_enc = nc.dram_tensor("enc", [128, B, H], FP16, kind="ExternalInput")
    d_awT = nc.dram_tensor("awT", [128, 2 * NK, H], FP16, kind="ExternalInput")
    d_owT = nc.dram_tensor("owT", [128, NK, VS], FP16, kind="ExternalInput")
    d_ob = nc.dram_tensor("ob", [128, VS], FP16, kind="ExternalInput")

    d_lp = nc.dram_tensor("lp", [BT, VS], FP32, kind="ExternalOutput")
    d_attn = nc.dram_tensor("attn", [T, B, S], FP32, kind="ExternalOutput")
    d_hl = nc.dram_tensor("hl", [128, NK, B], FP32, kind="ExternalOutput")

    with tile.TileContext(nc) as tc:
        _body(tc, nc, d_xT, d_wih, d_whh, d_bias, d_bnw, d_h0T, d_encT,
              d_enc, d_awT, d_owT, d_ob, d_lp, d_attn, d_hl)

    nc.compile()
    return nc


def _body(tc, nc, d_xT, d_wih, d_whh, d_bias, d_bnw, d_h0T, d_encT, d_enc,
          d_awT, d_owT, d_ob, d_lp, d_attn, d_hl):
    # Two SBUF allocation stacks: left holds the GRU-phase tensors (freed in
    # LIFO order as phases retire), right holds the late-phase weights and
    # the tensors that survive into the logits phase.
    with tc.tile_pool(name="persist", bufs=1) as per, \
         tc.tile_pool(name="dram", bufs=1, space="DRAM") as dramp, \
         tc.tile_pool(name="bw1", bufs=1, side="right") as bw1, \
         tc.tile_pool(name="bw2", bufs=1, side="right") as bw2:

        biasT = per.tile([128, 16], FP32)
        nc.sync.dma_start(biasT, d_bias.ap())
        bnw = per.tile([128, NK, B], FP32)
        nc.sync.dma_start(bnw, d_bnw.ap())
        h0f = per.tile([128, NK, B], FP32)
        nc.sync.dma_start(h0f, d_h0T.ap())
        ident = per.tile([128, 128], FP32)
        make_identity(nc, ident)

        # tiny warmup collective: pays the cold ncfw/CC-path cost during the
        # GRU so the real normalizer AllReduces run at the warm floor
        wu_in = dramp.tile([128, 1], FP32)
        wu_out = dramp.tile([128, 1], FP32)
        wu_s = per.tile([128, 1], FP32)
        nc.any.memset(wu_s, 0.0)
        nc.gpsimd.dma_start(wu_in, wu_s)
        nc.gpsimd.collective_compute(
            "AllReduce", ALU.add,
            replica_groups=[list(range(NCORES))],
            ins=[wu_in.opt()], outs=[wu_out.opt()])

        awT = bw1.tile([128, 2 * NK, H], FP16)
        nc.sync.dma_start(awT, d_awT.ap())
        ob = bw1.tile([128, VS], FP16)
        nc.sync.dma_start(ob, d_ob.ap())

        with tc.tile_pool(name="encp", bufs=1) as ep:
            encT = ep.tile([128, B, NK, S], FP16)
            nc.scalar.dma_start(encT, d_encT.ap())

            with tc.tile_pool(name="outs", bufs=1) as ot:
                # hidden states for all steps, transposed dense: [p, k, b, t]
                outsT = ot.tile([128, NK, B, T], FP32)

                with tc.tile_pool(name="p12", bufs=1) as p12:
                    xgT = p12.tile([128, NM, B, T], FP32)
                    whh = p12.tile([128, NK, 3 * H], FP16)

                    # ------------ P1: xgT = W_ih @ x.T (+ biases) ---------
                    with tc.tile_pool(name="gw1", bufs=1) as gw1, \
                         tc.tile_pool(name="ps1", bufs=4, space="PSUM") as ps1:
                        xT = gw1.tile([128, NK, BT], FP16)
                        wih = gw1.tile([128, NK, 3 * H], FP16)
                        for k in range(NK):
                            nc.scalar.dma_start(wih[:, k, :], d_wih.ap()[:, k, :])
                            nc.sync.dma_start(xT[:, k, :], d_xT.ap()[:, k, :])
                        nc.sync.dma_start(whh, d_whh.ap())

                        for m in range(NM):
                            bcol = m if m < 8 else 8 + (m - 8)
                            pshalf = [ps1.tile([128, 512], FP32, tag="xg",
                                               name=f"xg{m}_{h_}")
                                      for h_ in range(2)]
                            for k in range(NK):
                                for half in range(2):
                                    nc.tensor.matmul(
                                        pshalf[half],
                                        lhsT=wih[:, k, bass.ts(m, 128)],
                                        rhs=xT[:, k, bass.ts(half, 512)],
                                        start=(k == 0), stop=(k == NK - 1))
                            for half in range(2):
                                dst = xgT[:, m, 8 * half:8 * (half + 1), :]
                                nc.scalar.activation(
                                    dst,
                                    pshalf[half].rearrange("p (b t) -> p b t",
                                                           t=T),
                                    AF.Identity,
                                    bias=biasT[:, bcol:bcol + 1])

                    # late-phase weights: load during the GRU
                    owT = bw2.tile([128, NK, VS], FP16)
                    nc.sync.dma_start(owT, d_owT.ap())
                    encbAll = bw2.tile([128, B, H], FP16)
                    nc.scalar.dma_start(encbAll, d_enc.ap())
                    outs16 = bw2.tile([128, NK, B, T], FP16)
                    mixT = bw2.tile([128, NK, B, T], FP16)
                    out2T = bw2.tile([128, NK, B, T], FP16)

                    # ------------ P2: GRU recurrence ----------------------
                    # emission order r(0:4), n(8:12), z(4:8); h' = n + z*(h-n)
                    m_order = [0, 1, 2, 3, 8, 9, 10, 11, 4, 5, 6, 7]
                    with tc.tile_pool(name="gru", bufs=3) as gp, \
                         tc.tile_pool(name="hb", bufs=2) as hp, \
                         tc.tile_pool(name="psg", bufs=2, space="PSUM") as psg:
                        hbf = hp.tile([128, NK, B], FP16, tag="hbf")
                        nc.vector.tensor_copy(hbf, h0f)

                        for t in range(T):
                            psR = psg.tile([128, NK, B], FP32, tag="gr",
                                           name=f"gr{t}")
                            psN = psg.tile([128, NK, B], FP32, tag="gn",
                                           name=f"gn{t}")
                            psZ = psg.tile([128, NK, B], FP32, tag="gz",
                                           name=f"gz{t}")
                            pdst = {**{m: psR[:, m, :] for m in range(4)},
                                    **{m + 4: psZ[:, m, :] for m in range(4)},
                                    **{m + 8: psN[:, m, :] for m in range(4)}}
                            for m in m_order:
                                for k in range(NK):
                                    nc.tensor.matmul(
                                        pdst[m],
                                        lhsT=whh[:, k, bass.ts(m, 128)],
                                        rhs=hbf[:, k, :],
                                        start=(k == 0), stop=(k == NK - 1))

                            hprev = h0f if t == 0 else outsT[:, :, :, t - 1]

                            rpre = gp.tile([128, NK, B], FP32, tag="rpre")
                            nc.vector.tensor_add(rpre, psR,
                                                 xgT[:, 0:4, :, t])
                            rr = gp.tile([128, NK, B], FP32, tag="rr")
                            nc.scalar.activation(rr, rpre, AF.Sigmoid)

                            hnb = gp.tile([128, NK, B], FP32, tag="hnb")
                            nc.vector.tensor_add(hnb, psN, bnw)
                            npre = gp.tile([128, NK, B], FP32, tag="npre")
                            nc.vector.tensor_mul(npre, rr, hnb)
                            nsum = gp.tile([128, NK, B], FP32, tag="nsum")
                            nc.vector.tensor_add(nsum, npre,
                                                 xgT[:, 8:12, :, t])
                            nn = gp.tile([128, NK, B], FP32, tag="nn")
                            nc.scalar.activation(nn, nsum, AF.Tanh)
                            dd = gp.tile([128, NK, B], FP32, tag="dd")
                            nc.vector.tensor_sub(dd, hprev, nn)

                            zpre = gp.tile([128, NK, B], FP32, tag="zpre")
                            nc.vector.tensor_add(zpre, psZ,
                                                 xgT[:, 4:8, :, t])
                            zz = gp.tile([128, NK, B], FP32, tag="zz")
                            nc.scalar.activation(zz, zpre, AF.Sigmoid)
                            zd = gp.tile([128, NK, B], FP32, tag="zd")
                            nc.vector.tensor_mul(zd, zz, dd)

                            hbf = hp.tile([128, NK, B], FP16, tag="hbf")
                            nc.vector.tensor_add(hbf, nn, zd)
                            nc.vector.tensor_add(outsT[:, :, :, t], nn, zd)
                            nc.scalar.activation(outs16[:, :, :, t], hbf,
                                                 AF.Copy)

                nc.sync.dma_start(d_hl.ap(), outsT[:, :, :, T - 1])

                # ------------ P4: attention, three batched passes ----------
                with tc.tile_pool(name="att", bufs=3) as ap_, \
                     tc.tile_pool(name="atall", bufs=1) as alp, \
                     tc.tile_pool(name="pss", bufs=3, space="PSUM") as pss_p, \
                     tc.tile_pool(name="pst", bufs=2, space="PSUM") as pst_p, \
                     tc.tile_pool(name="psm", bufs=2, space="PSUM") as psm_p:

                    esAll = alp.tile([64, B, S], FP32)
                    ssumAll = alp.tile([64, B], FP32)
                    attnAll = alp.tile([64, B, S], FP32)
                    recAll = alp.tile([64, B], FP32)

                    # pass 1: scores + exp/rowsum per batch (PE -> ACT)
                    for b in range(B):
                        pss = pss_p.tile([64, S], FP32, tag="sc", name=f"sc{b}")
                        for k in range(NK):
                            nc.tensor.matmul(pss,
                                             lhsT=outs16[:, k, b, :],
                                             rhs=encT[:, b, k, :],
                                             start=(k == 0), stop=(k == NK - 1))
                        nc.scalar.activation(esAll[:, b, :], pss, AF.Exp,
                                             accum_out=ssumAll[:, b:b + 1])

                    # pass 2: batched normalize (free-dim broadcast of 1/sum)
                    nc.vector.reciprocal(recAll, ssumAll)
                    rec_b = recAll.rearrange("p (b o) -> p b o", o=1).broadcast_to((64, B, S))
                    nc.vector.tensor_mul(attnAll, esAll, rec_b)
                    nc.sync.dma_start(d_attn.ap(), attnAll)

                    # pass 3: transpose + mix (PE -> DVE -> PE), pipelined
                    for b in range(B):
                        pst = pst_p.tile([128, 64], FP32, tag="tr",
                                         name=f"tr{b}")
                        nc.tensor.transpose(pst, attnAll[:, b, :],
                                            ident[0:64, 0:64])
                        atT = ap_.tile([128, 64], FP16, tag="atT")
                        nc.vector.tensor_copy(atT, pst)
                        psm = psm_p.tile([128, NK, 64], FP32, tag="mx",
                                         name=f"mx{b}")
                        for m in range(NK):
                            nc.tensor.matmul(psm[:, m, :],
                                             lhsT=encbAll[:, b, bass.ts(m, 128)],
                                             rhs=atT, start=True, stop=True)
                        nc.vector.tensor_copy(mixT[:, :, b, :], psm)

        # re-sync cores so the normalizer AllReduces see minimal skew
        wu2_in = dramp.tile([128, 1], FP32)
        wu2_out = dramp.tile([128, 1], FP32)
        nc.gpsimd.dma_start(wu2_in, wu_s)
        nc.gpsimd.collective_compute(
            "AllReduce", ALU.add,
            replica_groups=[list(range(NCORES))],
            ins=[wu2_in.opt()], outs=[wu2_out.opt()])

        # ------------ P5: out2T = tanh(attn_W @ combinedT + b) -----------
        with tc.tile_pool(name="pso", bufs=3, space="PSUM") as pso_p:
            for m in range(NK):
                for half in range(2):
                    pso = pso_p.tile([128, 512], FP32, tag="o2")
                    for k in range(2 * NK):
                        src = mixT if k < NK else outs16
                        rhs = src[:, k % NK, 8 * half:8 * (half + 1), :]
                        nc.tensor.matmul(pso, lhsT=awT[:, k, bass.ts(m, 128)],
                                         rhs=rhs, start=(k == 0),
                                         stop=(k == 2 * NK - 1))
                    nc.scalar.activation(
                        out2T[:, m, 8 * half:8 * (half + 1), :],
                        pso.rearrange("p (b t) -> p b t", t=T),
                        AF.Tanh, bias=biasT[:, 12 + m:13 + m])

        # ------ P6/P7: logits, exp-sum stats, AllReduce, writeout --------
        # two stages of 4 row-tiles each so stage-0 writes overlap stage-1 MMs
        with tc.tile_pool(name="lg", bufs=1, side="right") as lgp, \
             tc.tile_pool(name="esc", bufs=1, side="right") as escp, \
             tc.tile_pool(name="stat", bufs=1, side="right") as stp, \
             tc.tile_pool(name="psl", bufs=8, space="PSUM") as psl_p, \
             tc.tile_pool(name="fin", bufs=2, side="right") as finp:
            logits = lgp.tile([128, RT, VS], FP16)

            NST = 2
            RPS = RT // NST
            for stage in range(NST):
                rts = range(RPS * stage, RPS * stage + RPS)
                Scol = stp.tile([128, RPS], FP32, tag=f"sc{stage}")
                for rt in rts:
                    psl = [psl_p.tile([128, VC], FP32, tag="lg",
                                      name=f"lg{rt}_{vc_}")
                           for vc_ in range(NVC)]
                    for k in range(NK):
                        for vc in range(NVC):
                            nc.tensor.matmul(
                                psl[vc],
                                lhsT=out2T[:, k, 2 * rt:2 * rt + 2, :],
                                rhs=owT[:, k, bass.ts(vc, VC)],
                                start=(k == 0), stop=(k == NK - 1))
                    for vc in range(NVC):
                        nc.vector.tensor_add(logits[:, rt, bass.ts(vc, VC)],
                                             psl[vc], ob[:, bass.ts(vc, VC)])
                    es = escp.tile([128, VS], FP16, tag="esc")
                    nc.scalar.activation(es, logits[:, rt, :], AF.Exp,
                                         accum_out=Scol[:, rt - RPS * stage:
                                                        rt - RPS * stage + 1])

                cc_in = dramp.tile([128, RPS], FP32, tag=f"ci{stage}")
                cc_out = dramp.tile([128, RPS], FP32, tag=f"co{stage}")
                nc.gpsimd.dma_start(cc_in, Scol)
                nc.gpsimd.collective_compute(
                    "AllReduce", ALU.add,
                    replica_groups=[list(range(NCORES))],
                    ins=[cc_in.opt()],
                    outs=[cc_out.opt()])
                Sg = stp.tile([128, RPS], FP32, tag=f"sg{stage}")
                nc.gpsimd.dma_start(Sg, cc_out)
                lse = stp.tile([128, RPS], FP32, tag=f"ls{stage}")
                nc.scalar.activation(lse, Sg, AF.Ln)
                nlse = stp.tile([128, RPS], FP32, tag=f"nl{stage}")
                nc.vector.tensor_scalar_mul(nlse, lse, -1.0)

                for rt in rts:
                    i = rt - RPS * stage
                    ofp = finp.tile([128, VS], FP32, tag="of")
                    if rt % 4 == 3:
                        nc.scalar.activation(ofp, logits[:, rt, :],
                                             AF.Identity,
                                             bias=nlse[:, i:i + 1])
                    else:
                        nc.vector.tensor_scalar_add(ofp, logits[:, rt, :],
                                                    nlse[:, i:i + 1])
                    q = nc.sync if rt % 2 == 0 else nc.scalar
                    q.dma_start(d_lp.ap()[bass.ts(rt, 128)], ofp)


_PROGRAM = None


def _get_program():
    global _PROGRAM
    if _PROGRAM is None:
        _patch_walrus_flags()
        _PROGRAM = _build_program()
    return _PROGRAM


def _pmajor(a, nk):
    """[nk*128, X...] -> [128, nk, X...] partition-major."""
    return np.ascontiguousarray(
        a.reshape((nk, 128) + a.shape[1:]).transpose(
            (1, 0) + tuple(range(2, a.ndim + 1))))


def _prep_inputs(inputs, encoder_hidden, encoder_outputs, emb, W_ih, W_hh,
                 b_ih, b_hh, attn_W, attn_b, out_W, out_b):
    """Host-side sharding/layout: returns per-core input maps."""
    f32 = np.float32
    dec = np.asarray(inputs)[:, :-1]
    x = np.asarray(emb, f32)[dec]                       # [B, T, H]
    x2 = x.reshape(BT, H)                               # rows b*T + t
    xT = _pmajor(np.ascontiguousarray(x2.T).astype(f16), NK)

    wih = _pmajor(np.ascontiguousarray(np.asarray(W_ih, f32).T).astype(f16), NK)
    whh = _pmajor(np.ascontiguousarray(np.asarray(W_hh, f32).T).astype(f16), NK)

    b_ih = np.asarray(b_ih, f32)
    b_hh = np.asarray(b_hh, f32)
    attn_b = np.asarray(attn_b, f32)
    bias = np.zeros((128, 16), f32)
    bias[:, 0:8] = (b_ih + b_hh)[:2 * H].reshape(8, 128).T
    bias[:, 8:12] = b_ih[2 * H:].reshape(NK, 128).T
    bias[:, 12:16] = attn_b.reshape(NK, 128).T
    bnw = np.repeat(b_hh[2 * H:].reshape(NK, 128).T[:, :, None], B, axis=2)
    bnw = np.ascontiguousarray(bnw, f32)                # [128, NK, B]

    h0 = np.asarray(encoder_hidden, f32)[0]             # [B, H]
    h0T = np.ascontiguousarray(
        h0.T.reshape(NK, 128, B).transpose(1, 0, 2), f32)

    enc = np.asarray(encoder_outputs, f32)              # [B, S, H]
    encT = np.ascontiguousarray(
        enc.transpose(0, 2, 1).reshape(B, NK, 128, S).transpose(2, 0, 1, 3)
    ).astype(f16)                                       # [128, B, NK, S]
    encB = np.ascontiguousarray(enc.transpose(1, 0, 2)).astype(f16)

    awT = _pmajor(np.ascontiguousarray(np.asarray(attn_W, f32).T).astype(f16),
                  2 * NK)

    out_W = np.asarray(out_W, f32)
    out_b = np.asarray(out_b, f32)

    common = dict(xT=xT, wih=wih, whh=whh, bias=bias, bnw=bnw, h0T=h0T,
                  encT=encT, enc=encB, awT=awT)
    in_maps = []
    for c in range(NCORES):
        sl = slice(c * VS, (c + 1) * VS)
        owT = _pmajor(np.ascontiguousarray(out_W[sl].T).astype(f16), NK)
        obt = np.ascontiguousarray(
            np.broadcast_to(out_b[sl].astype(f16), (128, VS)))
        in_maps.append(dict(common, owT=owT, ob=obt))
    return in_maps


def run_raw(inputs, **run_kwargs):
    """Run the SPMD kernel; returns ((log_probs, h_last, attn), BassKernelResults)."""
    nc = _get_program()
    in_maps = _prep_inputs(**inputs)
    res = run_bass_kernel_spmd(nc, in_maps, core_ids=list(range(NCORES)),
                               **run_kwargs)
    return _assemble(res.results), res


def _assemble(outs):
    lp = np.concatenate([outs[c]["lp"] for c in range(NCORES)], axis=1)
    log_probs = lp.reshape(B, T, V).astype(np.float32)

    attn = np.ascontiguousarray(
        np.asarray(outs[0]["attn"], np.float32).transpose(1, 0, 2))

    hl = np.asarray(outs[0]["hl"], np.float32)          # [128, NK, B]
    h_last = hl.transpose(2, 1, 0).reshape(B, H)[None]  # [1, B, H]

    return log_probs, h_last, attn


def kernel(**inputs):
    out, _ = run_raw(inputs)
    return out
